# revision 16
# baseline (speedup 1.0000x reference)
"""GCN (DGL GraphConv, norm='both', relu) on 8 Trainium2 NeuronCores.

Strategy (graph/data parallel, dst-sharded):
  - Sort edges by dst on host; core c owns dst nodes [c*12500, (c+1)*12500)
    and exactly the edges pointing into that range.  No halo exchange is
    needed: each core produces only its own output rows and gathers source
    features from a replicated bf16 feature table in its HBM.
  - The per-edge source rows are fetched with the batched Q7 `dma_gather`
    (int16 indices => the table is addressed in 4 quadrants of 32768 rows;
    each (128-dst-node block x quadrant) edge segment is padded to a
    multiple of 128 and groups of 8 blocks share one gather per quadrant).
  - Aggregation per block is a PE matmul: for each 128-edge chunk,
    A^T += M_chunk^T @ Sel_chunk, where Sel[e, d] =
    (localdst[e] == d) * (s_out[src_e] * s_in[dst_e]) is built in one fused
    DVE tensor_scalar (is_equal then mult, both scalars per-partition).
    The per-edge scale folds both degree normalizations, so no extra pass
    over the feature table is needed.
  - Then out_block = relu(ones^T @ bias + A @ W) via two f32 matmuls into
    PSUM and an ACT relu; one store per block.

Host does only index-domain preprocessing (degree counts, sorting,
layout/padding) plus the dtype cast of the feature table to bf16.
"""

import numpy as np
import ml_dtypes

P = 128
N_CORES = 8
QROWS = 32768  # int16 index range per table quadrant
GROUP = 8  # dst blocks per gather group

BF16 = np.dtype(ml_dtypes.bfloat16)

# set by test.py to collect a profile; harness path leaves these alone
TRACE = False
LAST_RESULT = None


def _build_schedule(src, dst, n_nodes, n_cores):
    """Host-side index preprocessing.

    Chunk order is (group, quadrant, block, k).  Returns (meta, per_core)
    where meta holds the compile-time schedule shared by all cores and
    per_core[c] holds gidx int16 [128, 8*Ktot] (dma_gather wrapped index
    layout) and ldsc f32 [128, 2*Ktot + 128]
    (localdst cols | scale cols | iota cols).
    """
    src = src.astype(np.int64)
    dst = dst.astype(np.int64)

    deg_out = np.bincount(src, minlength=n_nodes).astype(np.float64)
    deg_in = np.bincount(dst, minlength=n_nodes).astype(np.float64)
    s_out = 1.0 / np.sqrt(np.clip(deg_out, 1.0, None))
    s_in = 1.0 / np.sqrt(np.clip(deg_in, 1.0, None))

    assert n_nodes % n_cores == 0
    npc = n_nodes // n_cores
    blocks = (npc + P - 1) // P
    n_groups = (blocks + GROUP - 1) // GROUP
    nq = (n_nodes + QROWS - 1) // QROWS

    rows_list = [min(P, npc - b * P) for b in range(blocks)]

    core_of = dst // npc
    local = dst - core_of * npc
    blk = local // P
    grp = blk // GROUP
    quad = src // QROWS

    # order edges by (core, group, quadrant, block, dst)
    order = np.lexsort((dst, blk, quad, grp, core_of))
    ss = src[order]
    ds = dst[order]
    sc_e = (s_out[ss] * s_in[ds]).astype(np.float32)
    cq = quad[order]
    cb = (core_of * blocks + blk)[order]  # global block id

    # counts per (core, block, quadrant)
    key = cb * nq + cq
    cnt = np.bincount(key, minlength=n_cores * blocks * nq).reshape(
        n_cores, blocks, nq
    )
    # chunks per (block, quadrant): max over cores (shared SPMD program)
    K_bq = np.ceil(cnt.max(axis=0) / P).astype(np.int64)  # [blocks, nq]
    # every block needs >= 1 chunk so its PSUM tile gets initialised
    zero_blocks = K_bq.sum(axis=1) == 0
    K_bq[zero_blocks, 0] = 1

    # chunk layout in (group, quadrant, block, k) order
    chunk_cols = {}  # (b, q) -> first chunk index
    gathers = []  # (q, src_base, first_chunk, n_chunks) per (group, quadrant)
    t = 0
    for g in range(n_groups):
        bs = range(g * GROUP, min((g + 1) * GROUP, blocks))
        for q in range(nq):
            t0 = t
            for b in bs:
                if K_bq[b, q]:
                    chunk_cols[(b, q)] = t
                    t += int(K_bq[b, q])
            # cap gather size: >~2K indices per dma_gather crashes the Q7
            CAP = 8
            while t0 < t:
                nch = min(CAP, t - t0)
                gathers.append((q, q * QROWS, t0, nch))
                t0 += nch
    Ktot = t

    gather_of_chunk = {}
    for gi, (q, base, t0, nch) in enumerate(gathers):
        for tt in range(t0, t0 + nch):
            gather_of_chunk[tt] = gi
    # per-block chunk list: (gather_index, col_in_gather, global_col)
    block_chunks = []
    for b in range(blocks):
        lst = []
        for q in range(nq):
            if (b, q) in chunk_cols:
                t0 = chunk_cols[(b, q)]
                for k in range(int(K_bq[b, q])):
                    tt = t0 + k
                    gi = gather_of_chunk[tt]
                    lst.append((gi, tt - gathers[gi][2], tt))
        block_chunks.append(lst)

    # edge segment boundaries in the (core, group, quadrant, block) sort order
    segkey = ((core_of * n_groups + grp) * nq + quad) * blocks + blk
    seg_cnt = np.bincount(segkey, minlength=n_cores * n_groups * nq * blocks)
    starts = np.zeros(seg_cnt.size + 1, dtype=np.int64)
    np.cumsum(seg_cnt, out=starts[1:])

    def seg_range(c, b, q):
        kk = ((c * n_groups + b // GROUP) * nq + q) * blocks + b
        return starts[kk], starts[kk + 1]
    per_core = []
    for c in range(n_cores):
        gidx = np.zeros((16, 8 * Ktot), dtype=np.int16)
        lds = np.full((P, Ktot), -1.0, dtype=np.float32)
        scs = np.zeros((P, Ktot), dtype=np.float32)
        for b in range(blocks):
            base_node = c * npc + b * P
            for q in range(nq):
                if (b, q) not in chunk_cols:
                    continue
                t0 = chunk_cols[(b, q)]
                i0, i1 = seg_range(c, b, q)
                n = i1 - i0
                npad = int(K_bq[b, q]) * P
                iv = np.zeros(npad, dtype=np.int16)
                lv = np.full(npad, -1.0, dtype=np.float32)
                sv = np.zeros(npad, dtype=np.float32)
                iv[:n] = (ss[i0:i1] - q * QROWS).astype(np.int16)
                lv[:n] = (ds[i0:i1] - base_node).astype(np.float32)
                sv[:n] = sc_e[i0:i1]
                for k in range(int(K_bq[b, q])):
                    tt = t0 + k
                    sl = iv[k * P : (k + 1) * P]
                    # dma_gather wrap: idx i -> (partition i%16, col i//16)
                    gidx[:, 8 * tt : 8 * tt + 8] = sl.reshape(8, 16).T
                    lds[:, tt] = lv[k * P : (k + 1) * P]
                    scs[:, tt] = sv[k * P : (k + 1) * P]
        iota_cols = np.broadcast_to(np.arange(P, dtype=np.float32), (P, P))
        per_core.append(
            dict(
                gidx=np.tile(gidx, (8, 1)),
                ldsc=np.concatenate([lds, scs, iota_cols], axis=1).astype(
                    np.float32
                ),
            )
        )

    meta = dict(
        Ktot=Ktot,
        rows_list=rows_list,
        gathers=gathers,
        block_chunks=block_chunks,
        blocks=blocks,
        npc=npc,
    )
    return meta, per_core


def _build_program(n_nodes, n_feats, n_hidden, meta):
    import concourse.bass as bass  # noqa: F401
    import concourse.mybir as mybir
    import concourse.tile as tile
    from concourse.bacc import Bacc

    F, H = n_feats, n_hidden
    Ktot = meta["Ktot"]
    rows_list = meta["rows_list"]
    gathers = meta["gathers"]
    block_chunks = meta["block_chunks"]
    n_out = sum(rows_list)

    nc = Bacc()
    table = nc.declare_dram_parameter(
        "table", [n_nodes, F], mybir.dt.bfloat16, isOutput=False
    )
    gidx = nc.declare_dram_parameter(
        "gidx", [P, 8 * Ktot], mybir.dt.int16, isOutput=False
    )
    # [localdst cols | scale cols | 128 iota cols]
    ldsc = nc.declare_dram_parameter(
        "ldsc", [P, 2 * Ktot + P], mybir.dt.float32, isOutput=False
    )
    wmat = nc.declare_dram_parameter("wmat", [F, H], mybir.dt.float32, isOutput=False)
    # cols [0,H) = ones (lhsT of the bias outer product), cols [H,2H) = bias
    ob = nc.declare_dram_parameter("ob", [1, 2 * H], mybir.dt.float32, isOutput=False)
    out = nc.declare_dram_parameter("out", [n_out, H], mybir.dt.float32, isOutput=True)

    with tile.TileContext(nc) as tc:
        with (
            tc.tile_pool(name="const", bufs=1) as cpool,
            tc.tile_pool(name="ip", bufs=8) as ipool,
            tc.tile_pool(name="mp", bufs=8) as mpool,
            tc.tile_pool(name="sel", bufs=40) as selpool,
            tc.tile_pool(name="at", bufs=3) as atpool,
            tc.tile_pool(name="outp", bufs=16) as outpool,
            tc.tile_pool(name="psA", bufs=2, space="PSUM") as psA,
            tc.tile_pool(name="psO", bufs=2, space="PSUM") as psO,
        ):
            ldsc_t = cpool.tile([P, 2 * Ktot + P], mybir.dt.float32)
            nc.sync.dma_start(out=ldsc_t[:], in_=ldsc[:])
            w_t = cpool.tile([F, H], mybir.dt.float32)
            nc.sync.dma_start(out=w_t[:], in_=wmat[:])
            ob_t = cpool.tile([1, 2 * H], mybir.dt.float32)
            nc.sync.dma_start(out=ob_t[:], in_=ob[:])
            iota_ap = ldsc_t[:, 2 * Ktot : 2 * Ktot + P]

            # emit each gather right before the first block that consumes
            # it, so pool-slot demand follows consumption order
            first_block = {}
            for b, chunks in enumerate(block_chunks):
                for gi, _, _ in chunks:
                    first_block.setdefault(gi, b)
            by_block = {}
            for gi, b in first_block.items():
                by_block.setdefault(b, []).append(gi)
            m_tiles = [None] * len(gathers)

            def emit_gather(gi):
                q, src_base, t0, nch = gathers[gi]
                it = ipool.tile([P, 8 * nch], mybir.dt.int16, tag="i")
                nc.sync.dma_start(out=it[:], in_=gidx[:, 8 * t0 : 8 * (t0 + nch)])
                mt = mpool.tile([P, nch, F], mybir.dt.bfloat16, tag="m")
                hi = min(src_base + QROWS, n_nodes)
                nc.gpsimd.dma_gather(
                    out_ap=mt[:],
                    in_ap=table[src_base:hi, :],
                    idxs_ap=it[:],
                    num_idxs=nch * P,
                    num_idxs_reg=nch * P,
                    elem_size=F,
                )
                m_tiles[gi] = mt

            for b, chunks in enumerate(block_chunks):
                for gi in sorted(by_block.get(b, [])):
                    emit_gather(gi)
                pat = psA.tile([P, P], mybir.dt.float32)
                nch = len(chunks)
                for j, (gi, kk, tt) in enumerate(chunks):
                    sel = selpool.tile([P, P], mybir.dt.bfloat16, tag="sel")
                    nc.vector.tensor_scalar(
                        out=sel[:],
                        in0=iota_ap,
                        scalar1=ldsc_t[:, tt : tt + 1],
                        scalar2=ldsc_t[:, Ktot + tt : Ktot + tt + 1],
                        op0=mybir.AluOpType.is_equal,
                        op1=mybir.AluOpType.mult,
                    )
                    nc.tensor.matmul(
                        out=pat[:],
                        lhsT=m_tiles[gi][:, kk, :],
                        rhs=sel[:],
                        start=(j == 0),
                        stop=(j == nch - 1),
                    )
                at = atpool.tile([P, P], mybir.dt.float32, tag="at")
                nc.vector.tensor_copy(out=at[:], in_=pat[:])
                po = psO.tile([P, H], mybir.dt.float32)
                nc.tensor.matmul(
                    out=po[:], lhsT=ob_t[:1, :H], rhs=ob_t[:1, H:], start=True,
                    stop=False,
                )
                nc.tensor.matmul(
                    out=po[:], lhsT=at[:], rhs=w_t[:], start=False, stop=True
                )
                ot = outpool.tile([P, H], mybir.dt.float32, tag="ot")
                nc.scalar.activation(
                    out=ot[:], in_=po[:], func=mybir.ActivationFunctionType.Relu
                )
                rows = rows_list[b]
                nc.sync.dma_start(out=out[b * P : b * P + rows, :], in_=ot[:rows, :])
    nc.compile()
    return nc


def kernel(features, src, dst, weight, bias):
    global LAST_RESULT
    from concourse.bass_utils import run_bass_kernel_spmd

    features = np.asarray(features, dtype=np.float32)
    src = np.asarray(src, dtype=np.int32)
    dst = np.asarray(dst, dtype=np.int32)
    weight = np.asarray(weight, dtype=np.float32)
    bias = np.asarray(bias, dtype=np.float32)

    n_nodes, n_feats = features.shape
    n_hidden = weight.shape[1]

    meta, per_core = _build_schedule(src, dst, n_nodes, N_CORES)
    nc = _build_program(n_nodes, n_feats, n_hidden, meta)

    table_bf16 = features.astype(BF16)
    ob_arr = np.concatenate(
        [np.ones(n_hidden, np.float32), bias.astype(np.float32)]
    ).reshape(1, 2 * n_hidden)
    w_arr = weight.astype(np.float32)

    in_maps = []
    for c in range(N_CORES):
        in_maps.append(
            dict(
                table=table_bf16,
                gidx=per_core[c]["gidx"],
                ldsc=per_core[c]["ldsc"],
                wmat=w_arr,
                ob=ob_arr,
            )
        )

    res = run_bass_kernel_spmd(nc, in_maps, list(range(N_CORES)), trace=TRACE)
    LAST_RESULT = res

    npc = meta["npc"]
    out = np.concatenate(
        [res.results[c]["out"][:npc] for c in range(N_CORES)], axis=0
    )
    return out.astype(np.float32)


# revision 17
# speedup vs baseline: 1.3570x; 1.3570x over previous
"""GCN (DGL GraphConv, norm='both', relu) on 8 Trainium2 NeuronCores.

Strategy (graph/data parallel, dst-sharded):
  - Sort edges by dst on host; core c owns dst nodes [c*12500, (c+1)*12500)
    and exactly the edges pointing into that range.  No halo exchange is
    needed: each core produces only its own output rows and gathers source
    features from a replicated bf16 feature table in its HBM.
  - The per-edge source rows are fetched with the batched Q7 `dma_gather`
    (int16 indices => the table is addressed in 4 quadrants of 32768 rows;
    each (128-dst-node block x quadrant) edge segment is padded to a
    multiple of 128 and groups of 8 blocks share one gather per quadrant).
  - Aggregation per block is a PE matmul: for each 128-edge chunk,
    A^T += M_chunk^T @ Sel_chunk, where Sel[e, d] =
    (localdst[e] == d) * (s_out[src_e] * s_in[dst_e]) is built in one fused
    DVE tensor_scalar (is_equal then mult, both scalars per-partition).
    The per-edge scale folds both degree normalizations, so no extra pass
    over the feature table is needed.
  - Then out_block = relu(ones^T @ bias + A @ W) via two f32 matmuls into
    PSUM and an ACT relu; one store per block.

Host does only index-domain preprocessing (degree counts, sorting,
layout/padding) plus the dtype cast of the feature table to bf16.
"""

import numpy as np
import ml_dtypes

P = 128
N_CORES = 8
QROWS = 32768  # int16 index range per table quadrant
GROUP = 8  # dst blocks per gather group

BF16 = np.dtype(ml_dtypes.bfloat16)

# set by test.py to collect a profile; harness path leaves these alone
TRACE = False
LAST_RESULT = None


def _build_schedule(src, dst, n_nodes, n_cores):
    """Host-side index preprocessing.

    Chunk order is (group, quadrant, block, k).  Returns (meta, per_core)
    where meta holds the compile-time schedule shared by all cores and
    per_core[c] holds gidx int16 [128, 8*Ktot] (dma_gather wrapped index
    layout) and ldsc f32 [128, 2*Ktot + 128]
    (localdst cols | scale cols | iota cols).
    """
    src = src.astype(np.int64)
    dst = dst.astype(np.int64)

    deg_out = np.bincount(src, minlength=n_nodes).astype(np.float64)
    deg_in = np.bincount(dst, minlength=n_nodes).astype(np.float64)
    s_out = 1.0 / np.sqrt(np.clip(deg_out, 1.0, None))
    s_in = 1.0 / np.sqrt(np.clip(deg_in, 1.0, None))

    assert n_nodes % n_cores == 0
    npc = n_nodes // n_cores
    blocks = (npc + P - 1) // P
    n_groups = (blocks + GROUP - 1) // GROUP
    nq = (n_nodes + QROWS - 1) // QROWS

    rows_list = [min(P, npc - b * P) for b in range(blocks)]

    core_of = dst // npc
    local = dst - core_of * npc
    blk = local // P
    grp = blk // GROUP
    quad = src // QROWS

    # order edges by (core, group, quadrant, block, dst)
    order = np.lexsort((dst, blk, quad, grp, core_of))
    ss = src[order]
    ds = dst[order]
    sc_e = (s_out[ss] * s_in[ds]).astype(np.float32)
    cq = quad[order]
    cb = (core_of * blocks + blk)[order]  # global block id

    # counts per (core, block, quadrant)
    key = cb * nq + cq
    cnt = np.bincount(key, minlength=n_cores * blocks * nq).reshape(
        n_cores, blocks, nq
    )
    # chunks per (block, quadrant): max over cores (shared SPMD program)
    K_bq = np.ceil(cnt.max(axis=0) / P).astype(np.int64)  # [blocks, nq]
    # every block needs >= 1 chunk so its PSUM tile gets initialised
    zero_blocks = K_bq.sum(axis=1) == 0
    K_bq[zero_blocks, 0] = 1

    # chunk layout in (group, quadrant, block, k) order
    chunk_cols = {}  # (b, q) -> first chunk index
    gathers = []  # (q, src_base, first_chunk, n_chunks) per (group, quadrant)
    t = 0
    for g in range(n_groups):
        bs = range(g * GROUP, min((g + 1) * GROUP, blocks))
        for q in range(nq):
            t0 = t
            for b in bs:
                if K_bq[b, q]:
                    chunk_cols[(b, q)] = t
                    t += int(K_bq[b, q])
            # cap gather size: >~2K indices per dma_gather crashes the Q7
            CAP = 8
            while t0 < t:
                nch = min(CAP, t - t0)
                gathers.append((q, q * QROWS, t0, nch))
                t0 += nch
    Ktot = t

    gather_of_chunk = {}
    for gi, (q, base, t0, nch) in enumerate(gathers):
        for tt in range(t0, t0 + nch):
            gather_of_chunk[tt] = gi
    # per-block chunk list: (gather_index, col_in_gather, global_col)
    block_chunks = []
    for b in range(blocks):
        lst = []
        for q in range(nq):
            if (b, q) in chunk_cols:
                t0 = chunk_cols[(b, q)]
                for k in range(int(K_bq[b, q])):
                    tt = t0 + k
                    gi = gather_of_chunk[tt]
                    lst.append((gi, tt - gathers[gi][2], tt))
        block_chunks.append(lst)

    # edge segment boundaries in the (core, group, quadrant, block) sort order
    segkey = ((core_of * n_groups + grp) * nq + quad) * blocks + blk
    seg_cnt = np.bincount(segkey, minlength=n_cores * n_groups * nq * blocks)
    starts = np.zeros(seg_cnt.size + 1, dtype=np.int64)
    np.cumsum(seg_cnt, out=starts[1:])

    def seg_range(c, b, q):
        kk = ((c * n_groups + b // GROUP) * nq + q) * blocks + b
        return starts[kk], starts[kk + 1]
    per_core = []
    for c in range(n_cores):
        gidx = np.zeros((16, 8 * Ktot), dtype=np.int16)
        lds = np.full((P, Ktot), -1.0, dtype=np.float32)
        scs = np.zeros((P, Ktot), dtype=np.float32)
        for b in range(blocks):
            base_node = c * npc + b * P
            for q in range(nq):
                if (b, q) not in chunk_cols:
                    continue
                t0 = chunk_cols[(b, q)]
                i0, i1 = seg_range(c, b, q)
                n = i1 - i0
                npad = int(K_bq[b, q]) * P
                iv = np.zeros(npad, dtype=np.int16)
                lv = np.full(npad, -1.0, dtype=np.float32)
                sv = np.zeros(npad, dtype=np.float32)
                iv[:n] = (ss[i0:i1] - q * QROWS).astype(np.int16)
                lv[:n] = (ds[i0:i1] - base_node).astype(np.float32)
                sv[:n] = sc_e[i0:i1]
                for k in range(int(K_bq[b, q])):
                    tt = t0 + k
                    sl = iv[k * P : (k + 1) * P]
                    # dma_gather wrap: idx i -> (partition i%16, col i//16)
                    gidx[:, 8 * tt : 8 * tt + 8] = sl.reshape(8, 16).T
                    lds[:, tt] = lv[k * P : (k + 1) * P]
                    scs[:, tt] = sv[k * P : (k + 1) * P]
        iota_cols = np.broadcast_to(np.arange(P, dtype=np.float32), (P, P))
        per_core.append(
            dict(
                gidx=np.tile(gidx, (8, 1)),
                ldsc=np.concatenate([lds, scs, iota_cols], axis=1).astype(
                    np.float32
                ),
            )
        )

    meta = dict(
        Ktot=Ktot,
        rows_list=rows_list,
        gathers=gathers,
        block_chunks=block_chunks,
        blocks=blocks,
        npc=npc,
    )
    return meta, per_core


def _build_program(n_nodes, n_feats, n_hidden, meta):
    import concourse.bass as bass  # noqa: F401
    import concourse.mybir as mybir
    import concourse.tile as tile
    from concourse.bacc import Bacc

    F, H = n_feats, n_hidden
    Ktot = meta["Ktot"]
    rows_list = meta["rows_list"]
    gathers = meta["gathers"]
    block_chunks = meta["block_chunks"]
    n_out = sum(rows_list)

    nc = Bacc(num_swdge_queues=4)
    table = nc.declare_dram_parameter(
        "table", [n_nodes, F], mybir.dt.bfloat16, isOutput=False
    )
    gidx = nc.declare_dram_parameter(
        "gidx", [P, 8 * Ktot], mybir.dt.int16, isOutput=False
    )
    # [localdst cols | scale cols | 128 iota cols]
    ldsc = nc.declare_dram_parameter(
        "ldsc", [P, 2 * Ktot + P], mybir.dt.float32, isOutput=False
    )
    wmat = nc.declare_dram_parameter("wmat", [F, H], mybir.dt.float32, isOutput=False)
    # cols [0,H) = ones (lhsT of the bias outer product), cols [H,2H) = bias
    ob = nc.declare_dram_parameter("ob", [1, 2 * H], mybir.dt.float32, isOutput=False)
    out = nc.declare_dram_parameter("out", [n_out, H], mybir.dt.float32, isOutput=True)

    with tile.TileContext(nc) as tc:
        with (
            tc.tile_pool(name="const", bufs=1) as cpool,
            tc.tile_pool(name="ip", bufs=8) as ipool,
            tc.tile_pool(name="mp", bufs=8) as mpool,
            tc.tile_pool(name="sel", bufs=40) as selpool,
            tc.tile_pool(name="at", bufs=3) as atpool,
            tc.tile_pool(name="outp", bufs=16) as outpool,
            tc.tile_pool(name="psA", bufs=2, space="PSUM") as psA,
            tc.tile_pool(name="psO", bufs=2, space="PSUM") as psO,
        ):
            ldsc_t = cpool.tile([P, 2 * Ktot + P], mybir.dt.float32)
            nc.sync.dma_start(out=ldsc_t[:], in_=ldsc[:])
            w_t = cpool.tile([F, H], mybir.dt.float32)
            nc.sync.dma_start(out=w_t[:], in_=wmat[:])
            ob_t = cpool.tile([1, 2 * H], mybir.dt.float32)
            nc.sync.dma_start(out=ob_t[:], in_=ob[:])
            iota_ap = ldsc_t[:, 2 * Ktot : 2 * Ktot + P]

            # emit each gather right before the first block that consumes
            # it, so pool-slot demand follows consumption order
            first_block = {}
            for b, chunks in enumerate(block_chunks):
                for gi, _, _ in chunks:
                    first_block.setdefault(gi, b)
            by_block = {}
            for gi, b in first_block.items():
                by_block.setdefault(b, []).append(gi)
            m_tiles = [None] * len(gathers)

            def emit_gather(gi):
                q, src_base, t0, nch = gathers[gi]
                it = ipool.tile([P, 8 * nch], mybir.dt.int16, tag="i")
                nc.sync.dma_start(out=it[:], in_=gidx[:, 8 * t0 : 8 * (t0 + nch)])
                mt = mpool.tile([P, nch, F], mybir.dt.bfloat16, tag="m")
                hi = min(src_base + QROWS, n_nodes)
                nc.gpsimd.dma_gather(
                    out_ap=mt[:],
                    in_ap=table[src_base:hi, :],
                    idxs_ap=it[:],
                    num_idxs=nch * P,
                    num_idxs_reg=nch * P,
                    elem_size=F,
                    queue_num=gi % 4,
                )
                m_tiles[gi] = mt

            for b, chunks in enumerate(block_chunks):
                for gi in sorted(by_block.get(b, [])):
                    emit_gather(gi)
                pat = psA.tile([P, P], mybir.dt.float32)
                nch = len(chunks)
                for j, (gi, kk, tt) in enumerate(chunks):
                    sel = selpool.tile([P, P], mybir.dt.bfloat16, tag="sel")
                    nc.vector.tensor_scalar(
                        out=sel[:],
                        in0=iota_ap,
                        scalar1=ldsc_t[:, tt : tt + 1],
                        scalar2=ldsc_t[:, Ktot + tt : Ktot + tt + 1],
                        op0=mybir.AluOpType.is_equal,
                        op1=mybir.AluOpType.mult,
                    )
                    nc.tensor.matmul(
                        out=pat[:],
                        lhsT=m_tiles[gi][:, kk, :],
                        rhs=sel[:],
                        start=(j == 0),
                        stop=(j == nch - 1),
                    )
                at = atpool.tile([P, P], mybir.dt.float32, tag="at")
                nc.vector.tensor_copy(out=at[:], in_=pat[:])
                po = psO.tile([P, H], mybir.dt.float32)
                nc.tensor.matmul(
                    out=po[:], lhsT=ob_t[:1, :H], rhs=ob_t[:1, H:], start=True,
                    stop=False,
                )
                nc.tensor.matmul(
                    out=po[:], lhsT=at[:], rhs=w_t[:], start=False, stop=True
                )
                ot = outpool.tile([P, H], mybir.dt.float32, tag="ot")
                nc.scalar.activation(
                    out=ot[:], in_=po[:], func=mybir.ActivationFunctionType.Relu
                )
                rows = rows_list[b]
                nc.sync.dma_start(out=out[b * P : b * P + rows, :], in_=ot[:rows, :])
    nc.compile()
    return nc


def kernel(features, src, dst, weight, bias):
    global LAST_RESULT
    from concourse.bass_utils import run_bass_kernel_spmd

    features = np.asarray(features, dtype=np.float32)
    src = np.asarray(src, dtype=np.int32)
    dst = np.asarray(dst, dtype=np.int32)
    weight = np.asarray(weight, dtype=np.float32)
    bias = np.asarray(bias, dtype=np.float32)

    n_nodes, n_feats = features.shape
    n_hidden = weight.shape[1]

    meta, per_core = _build_schedule(src, dst, n_nodes, N_CORES)
    nc = _build_program(n_nodes, n_feats, n_hidden, meta)

    table_bf16 = features.astype(BF16)
    ob_arr = np.concatenate(
        [np.ones(n_hidden, np.float32), bias.astype(np.float32)]
    ).reshape(1, 2 * n_hidden)
    w_arr = weight.astype(np.float32)

    in_maps = []
    for c in range(N_CORES):
        in_maps.append(
            dict(
                table=table_bf16,
                gidx=per_core[c]["gidx"],
                ldsc=per_core[c]["ldsc"],
                wmat=w_arr,
                ob=ob_arr,
            )
        )

    res = run_bass_kernel_spmd(nc, in_maps, list(range(N_CORES)), trace=TRACE)
    LAST_RESULT = res

    npc = meta["npc"]
    out = np.concatenate(
        [res.results[c]["out"][:npc] for c in range(N_CORES)], axis=0
    )
    return out.astype(np.float32)
